# revision 15
# baseline (speedup 1.0000x reference)
"""MeshPool kernel for Trainium2: per-mesh edge scoring, exact top-K selection,
order-preserving gather.  Data-parallel over B=16 meshes on 8 NeuronCores
(2 meshes per core).

v2 pipeline (replaces the GPSIMD ap_gather backend, ~110us/call, with a
descriptor-DMA gather, ~3us/chunk):

  1. Plain DMA load x -> 2 SBUF tiles [128, 9216] f32 (4 half-tile DMAs).
  2. score[e] = sum_c x[c,e]^2: ACT Square chunks + PE ones-matmul into a
     [1, 512] PSUM strip (fp32 exact); DVE bounces each strip to SBUF and
     Sync DMAs it to a DRAM scratch row score_d[9216].  Tail chunk is
     multiplied by a host 0/1 mask so invalid edges score 0.
  3. Wrapped-16 redistribution: srep[q, f] = score[16f+q]; 16 single-partition
     DMAs read score_d with stride-16; 3 doubling DMAs replicate to [128, 576].
  4. Exact K-th threshold via 7 levels of 8-ary histogram search.  Final bin
     width ~1.5e-5, far under the min adjacent-score gap, so count(>=T) == K.
  5. mask * iota(e+1); GPSIMD sparse_gather compacts to the 4096 kept edges
     (ascending).  The gather needs the DOUBLED interleaved list (2e, 2e+1) in
     wrapped-16 int16 [16, 512]: position 2j+h of the doubled list lives at
     [2*(j%8)+h, 2*(j//16) + (j%16)//8], so rows q'=2u+h pull from sg rows u
     (even cols) and 8+u (odd cols).  Two [16,16] permutation matmuls regroup
     the rows; two DVE adds (+row parity) write even/odd column planes.
  6. XBAR DMA-transpose (u16) of each x block into a token tile T:
     T[p, s*512 + b*256 + 2w] = u16 w of x_b[:, e]-column for 2e+h = 128s+p.
     Token idx = 2e+h sits at partition idx%128, rank idx//128, 512B each.
  7. nc.gpsimd.dma_gather (SBUF-source, transpose): descriptor DMA pulls token
     idx_i and 16-bit-transposes it into dst[:, :, i].  With the interleaved
     (2e, 2e+1) idx list the u16 halves land adjacently, so the dst IS the
     final fp32 [128, 2, cols] output tile: plane 0 = channels 0..127,
     plane 1 = 128..255.  4 chunks of 1024 columns; plain DMAs store each
     chunk straight to DRAM.
"""

import numpy as np

B, C, E, K = 16, 256, 9216, 4096
NCORES = 8
MPC = B // NCORES            # meshes per core
P = 128                      # partitions / channel block
NBLK = C // P                # channel blocks per mesh
CHUNK = 512
NCHUNK = E // CHUNK
TAIL = E - CHUNK             # 8704; all invalid edges live in the last chunk
W0 = 16                      # wrap width (sparse_gather / dma_gather idx ISA)
F1 = E // W0                 # 576 wrapped score columns
SGO = K // W0                # 256 compacted edge columns
SG2 = 2 * K // W0            # 512 doubled idx columns
NIDX = 2 * K                 # 8192 gather indices per mesh
GCH = 16                     # gather chunks per mesh (SWDGE ring caps descs/inst)
GIDX = NIDX // GCH           # 512 indices per gather
GCOLS = K // GCH             # 256 fp32 output columns per gather
GGRP = 4                     # gather chunks per output tile / store group
W1CUT = 12                   # chunks >= W1CUT read the rank-16 window, idx-2048
W1OFF = 2048                 # idx offset of the second window (16 ranks * 128)
RANKN = 2 * E // P           # 144 token ranks (512 B each)
HIST_LO = 240.0              # static threshold bracket; K-th score ~257
HIST_W0 = 32.0               # HIST_HI = 272
NLEV = 7                     # 8-ary levels; final width 32/8^7 ~ 1.5e-5

_CACHE = {}


def _build_program():
    import concourse.bacc as bacc
    import concourse.mybir as mybir
    import concourse.tile as tile
    from contextlib import ExitStack

    dt = mybir.dt
    op = mybir.AluOpType
    f32 = dt.float32
    i16 = dt.int16

    nc = bacc.Bacc()

    x_io = nc.dram_tensor("x", [MPC, C, E], f32, kind="ExternalInput")
    tailm_io = nc.dram_tensor("tailmask", [MPC, P, CHUNK], f32, kind="ExternalInput")
    ones_io = nc.dram_tensor("onesT", [P, P], f32, kind="ExternalInput")
    iotag_io = nc.dram_tensor("iota_g", [P, 1], f32, kind="ExternalInput")   # p // 16
    grp_io = nc.dram_tensor("grpind", [P, 8], f32, kind="ExternalInput")     # onehot(p//16)
    t1_io = nc.dram_tensor("t_lev1", [P, 1], f32, kind="ExternalInput")      # lo0+(p//16)*wb0
    iota1_io = nc.dram_tensor("iota1w", [W0, F1], f32, kind="ExternalInput") # 16f+q+1
    perm1_io = nc.dram_tensor("perm1", [W0, W0], f32, kind="ExternalInput")  # p==q'//2
    perm2_io = nc.dram_tensor("perm2", [W0, W0], f32, kind="ExternalInput")  # p==8+q'//2
    rowp_io = nc.dram_tensor("rowpar", [W0, 1], f32, kind="ExternalInput")   # q'%2
    out_io = nc.dram_tensor("out", [MPC, C, K], f32, kind="ExternalOutput")
    nf_io = nc.dram_tensor("nf", [MPC, 1], dt.uint32, kind="ExternalOutput")

    with tile.TileContext(nc) as tc, ExitStack() as ctx:
        constp = ctx.enter_context(tc.tile_pool(name="const", bufs=1))
        xpool = ctx.enter_context(tc.tile_pool(name="xb", bufs=2))
        sqpool = ctx.enter_context(tc.tile_pool(name="sqc", bufs=4))
        psump = ctx.enter_context(tc.tile_pool(name="ps", bufs=6, space="PSUM"))
        psmall = ctx.enter_context(tc.tile_pool(name="psm", bufs=2, space="PSUM"))
        dramp = ctx.enter_context(tc.tile_pool(name="scored", bufs=2, space="DRAM"))
        bouncep = ctx.enter_context(tc.tile_pool(name="bounce", bufs=4))
        tokp = ctx.enter_context(tc.tile_pool(name="tok", bufs=1))
        goutp = ctx.enter_context(tc.tile_pool(name="gout", bufs=2))
        smallp = ctx.enter_context(tc.tile_pool(name="small", bufs=1))

        ones_sb = constp.tile([P, P], f32, name="ones_sb")
        nc.sync.dma_start(ones_sb[:], ones_io[:])
        iotag_sb = constp.tile([P, 1], f32, name="iotag_sb")
        nc.sync.dma_start(iotag_sb[:], iotag_io[:])
        grp_sb = constp.tile([P, 8], f32, name="grp_sb")
        nc.sync.dma_start(grp_sb[:], grp_io[:])
        t1_sb = constp.tile([P, 1], f32, name="t1_sb")
        nc.sync.dma_start(t1_sb[:], t1_io[:])
        iota1_sb = constp.tile([W0, F1], f32, name="iota1_sb")
        nc.sync.dma_start(iota1_sb[:], iota1_io[:])
        perm1_sb = constp.tile([W0, W0], f32, name="perm1_sb")
        nc.sync.dma_start(perm1_sb[:], perm1_io[:])
        perm2_sb = constp.tile([W0, W0], f32, name="perm2_sb")
        nc.sync.dma_start(perm2_sb[:], perm2_io[:])
        rowp_sb = constp.tile([W0, 1], f32, name="rowp_sb")
        nc.sync.dma_start(rowp_sb[:], rowp_io[:])
        tailm_sb = []
        for m in range(MPC):
            tm = constp.tile([P, CHUNK], f32, name=f"tailm_sb{m}")
            nc.sync.dma_start(tm[:], tailm_io[m, :, :])
            tailm_sb.append(tm)

        state = [dict() for _ in range(MPC)]

        def emit_load(m):
            xblk = []
            for blk in range(NBLK):
                xt = xpool.tile([P, E], f32, name=f"x_m{m}b{blk}", tag="xb")
                half = E // 2
                nc.sync.dma_start(xt[:, 0:half], x_io[m, blk * P:(blk + 1) * P, 0:half])
                nc.sync.dma_start(xt[:, half:E], x_io[m, blk * P:(blk + 1) * P, half:E])
                xblk.append(xt)
            state[m]["xblk"] = xblk

        def emit_xbar_half(m, H):
            # transpose x cols [H*E/2, (H+1)*E/2) of both blocks (scalar HWDGE)
            xblk = state[m]["xblk"]
            if "T" not in state[m]:
                state[m]["T"] = tokp.tile([P, RANKN, 256], i16,
                                          name=f"tok_m{m}", tag="tok")
            T = state[m]["T"]
            half = E // 2
            RH = RANKN // 2
            for blk in range(NBLK):
                nc.scalar.dma_start_transpose(
                    T[:, H * RH:(H + 1) * RH, blk * P:(blk + 1) * P],
                    xblk[blk][:, H * half:(H + 1) * half].bitcast(i16),
                )

        def emit_score(m, with_xbar):
            xblk = state[m]["xblk"]
            score_d = dramp.tile([1, E], f32, name=f"scored_m{m}", tag="sd")
            for ch in range(NCHUNK):
                ps = psump.tile([1, CHUNK], f32, name=f"ps_m{m}c{ch}", tag="ps")
                for blk in range(NBLK):
                    xs = xblk[blk][:, ch * CHUNK:(ch + 1) * CHUNK]
                    sqc = sqpool.tile([P, CHUNK], f32, name=f"sq_m{m}c{ch}b{blk}",
                                      tag="sqc")
                    if blk == 0:
                        nc.scalar.square(sqc[:], xs)
                    else:
                        nc.vector.tensor_tensor(sqc[:], xs, xs, op.mult)
                    if ch == NCHUNK - 1:
                        nc.vector.tensor_tensor(sqc[:], sqc[:], tailm_sb[m][:],
                                                op.mult)
                    nc.tensor.matmul(ps[:], ones_sb[:, 0:1], sqc[:],
                                     start=(blk == 0), stop=(blk == NBLK - 1))
                bnc = bouncep.tile([1, CHUNK], f32, name=f"bnc_m{m}c{ch}", tag="bnc")
                nc.vector.tensor_copy(bnc[:], ps[:])
                nc.sync.dma_start(score_d[0:1, ch * CHUNK:(ch + 1) * CHUNK], bnc[:])
                if with_xbar and ch == NCHUNK // 2 - 1:
                    emit_xbar_half(m, 0)
            if with_xbar:
                emit_xbar_half(m, 1)
            state[m]["score_d"] = score_d

        def emit_select_a(m):
            score_d = state[m]["score_d"]
            # wrapped-16 redistribution: srep[q, f] = score[16f+q]
            srep = smallp.tile([P, F1], f32, name=f"srep_m{m}", tag="srep")
            s_w = score_d[0:1, :].rearrange("p (f s) -> p s f", s=W0)  # [1, 16, 576]
            for q in range(W0):
                eng = nc.sync if q % 2 == 0 else nc.scalar
                eng.dma_start(srep[q:q + 1, :], s_w[:, q, :])
            nc.sync.dma_start(srep[W0:2 * W0, :], srep[0:W0, :])
            nc.sync.dma_start(srep[2 * W0:4 * W0, :], srep[0:2 * W0, :])
            nc.sync.dma_start(srep[4 * W0:8 * W0, :], srep[0:4 * W0, :])
            state[m]["srep"] = srep

        def emit_select_b(m):
            srep = state[m]["srep"]
            # 8-ary histogram threshold search; state pair = [lo, wb]
            pair = smallp.tile([1, 2], f32, name=f"pair_m{m}", tag="pair")
            nc.vector.memset(pair[:, 0:1], HIST_LO)
            nc.vector.memset(pair[:, 1:2], HIST_W0 / 8.0)
            ge8 = smallp.tile([P, F1], dt.float8e4, name=f"ge8_m{m}", tag="ge8")
            junk8 = smallp.tile([1, 8], f32, name=f"junk8_m{m}", tag="junk8")
            for lev in range(NLEV):
                if lev == 0:
                    t_ap = t1_sb
                else:
                    tb = psmall.tile([P, 2], f32, name=f"tb_m{m}l{lev}", tag="psm")
                    nc.tensor.matmul(tb[:], ones_sb[0:1, :], pair[:],
                                     start=True, stop=True)
                    t_ap = smallp.tile([P, 1], f32, name=f"tap_m{m}l{lev}", tag="tap")
                    nc.vector.scalar_tensor_tensor(t_ap[:], iotag_sb[:], tb[:, 1:2],
                                                   tb[:, 0:1], op.mult, op.add)
                cnt = smallp.tile([P, 1], f32, name=f"cnt_m{m}l{lev}", tag="cnt")
                nc.vector.tensor_scalar(ge8[:], srep[:], t_ap[:, 0:1], None,
                                        op.is_ge, op1=op.add, accum_out=cnt[:])
                cnt8r = psmall.tile([1, 8], f32, name=f"cnt8_m{m}l{lev}", tag="psm")
                nc.tensor.matmul(cnt8r[:], cnt[:], grp_sb[:], start=True, stop=True)
                # DVE-local tail: s8 = #bins with doubled-count >= 2K (monotone)
                s8 = smallp.tile([1, 1], f32, name=f"s8_m{m}l{lev}", tag="s8")
                nc.vector.tensor_scalar(junk8[:], cnt8r[:], float(K), None,
                                        op.is_ge, op1=op.add, accum_out=s8[:])
                gstar = smallp.tile([1, 1], f32, name=f"gs_m{m}l{lev}", tag="gs")
                nc.vector.tensor_scalar(gstar[:], s8[:], 1.0, None, op.subtract)
                step = smallp.tile([1, 1], f32, name=f"step_m{m}l{lev}", tag="step")
                nc.vector.tensor_tensor(step[:], pair[:, 1:2], gstar[:], op.mult)
                nc.vector.tensor_tensor(pair[:, 0:1], pair[:, 0:1], step[:], op.add)
                if lev != NLEV - 1:
                    nc.vector.tensor_scalar(pair[:, 1:2], pair[:, 1:2], 0.125, None,
                                            op.mult)

            # select + compact (doubled sequence)
            t16 = psmall.tile([W0, 1], f32, name=f"t16_m{m}", tag="psm")
            nc.tensor.matmul(t16[:], ones_sb[0:1, 0:W0], pair[:, 0:1],
                             start=True, stop=True)
            m2 = smallp.tile([W0, F1], f32, name=f"m2_m{m}", tag="m2")
            nc.vector.tensor_scalar(m2[:], srep[0:W0, :], t16[:, 0:1], None, op.is_ge)
            nc.vector.tensor_scalar(m2[:], m2[:], 2.0, -1.0, op.mult, op1=op.add)
            sp2 = smallp.tile([W0, F1], f32, name=f"sp2_m{m}", tag="sp2")
            nc.vector.tensor_tensor(sp2[:], m2[:], iota1_sb[:], op.mult)
            sg2 = smallp.tile([W0, SGO], f32, name=f"sg2_m{m}", tag="sg2")
            nfs = smallp.tile([1, 1], dt.uint32, name=f"nfs_m{m}", tag="nfs")
            nc.gpsimd.sparse_gather(sg2[:], sp2[:], num_found=nfs[:])
            # s2 = 2*(e+1) - 2 = 2e; doubled idx list via row-permute matmuls
            s2 = smallp.tile([W0, SGO], f32, name=f"s2_m{m}", tag="s2")
            nc.vector.tensor_scalar(s2[:], sg2[:], 2.0, -2.0, op.mult, op1=op.add)
            pe1 = psmall.tile([W0, SGO], f32, name=f"pe1_m{m}", tag="psm")
            nc.tensor.matmul(pe1[:], perm1_sb[:], s2[:], start=True, stop=True)
            pe2 = psmall.tile([W0, SGO], f32, name=f"pe2_m{m}", tag="psm")
            nc.tensor.matmul(pe2[:], perm2_sb[:], s2[:], start=True, stop=True)
            idx128 = smallp.tile([P, SGO, 2], i16, name=f"idx128_m{m}", tag="idx128")
            nc.vector.tensor_scalar(idx128[0:W0, :, 0:1], pe1[:].rearrange("p (a b) -> p a b", b=1),
                                    rowp_sb[:, 0:1], None, op.add)
            nc.vector.tensor_scalar(idx128[0:W0, :, 1:2], pe2[:].rearrange("p (a b) -> p a b", b=1),
                                    rowp_sb[:, 0:1], None, op.add)
            nc.gpsimd.dma_start(idx128[W0:2 * W0], idx128[0:W0])
            nc.gpsimd.dma_start(idx128[2 * W0:4 * W0], idx128[0:2 * W0])
            nc.gpsimd.dma_start(idx128[4 * W0:8 * W0], idx128[0:4 * W0])
            # second-window idx list (chunks >= W1CUT): value - W1OFF, fp32 path
            ntail = SGO - W1CUT * (GIDX // 32)          # 64 pair-cols
            s2b = smallp.tile([W0, ntail], f32, name=f"s2b_m{m}", tag="s2b")
            nc.vector.tensor_scalar(s2b[:], sg2[:, SGO - ntail:SGO], 2.0,
                                    -2.0 - float(W1OFF), op.mult, op1=op.add)
            pe1b = psmall.tile([W0, ntail], f32, name=f"pe1b_m{m}", tag="psm")
            nc.tensor.matmul(pe1b[:], perm1_sb[:], s2b[:], start=True, stop=True)
            pe2b = psmall.tile([W0, ntail], f32, name=f"pe2b_m{m}", tag="psm")
            nc.tensor.matmul(pe2b[:], perm2_sb[:], s2b[:], start=True, stop=True)
            idxw1 = smallp.tile([P, ntail, 2], i16, name=f"idxw1_m{m}", tag="idxw1")
            nc.vector.tensor_scalar(idxw1[0:W0, :, 0:1],
                                    pe1b[:].rearrange("p (a b) -> p a b", b=1),
                                    rowp_sb[:, 0:1], None, op.add)
            nc.vector.tensor_scalar(idxw1[0:W0, :, 1:2],
                                    pe2b[:].rearrange("p (a b) -> p a b", b=1),
                                    rowp_sb[:, 0:1], None, op.add)
            nc.gpsimd.dma_start(idxw1[W0:2 * W0], idxw1[0:W0])
            nc.gpsimd.dma_start(idxw1[2 * W0:4 * W0], idxw1[0:2 * W0])
            nc.gpsimd.dma_start(idxw1[4 * W0:8 * W0], idxw1[0:4 * W0])
            state[m]["idx128"] = idx128
            state[m]["idxw1"] = idxw1
            state[m]["nfs"] = nfs

        def emit_gathers(m, grp):
            # chunks [grp*GGRP, (grp+1)*GGRP) into one chunk-major tile
            T = state[m]["T"]
            idx128 = state[m]["idx128"]
            idxw1 = state[m]["idxw1"]
            T2 = T[:].rearrange("p a b -> p (a b)")
            T2w1 = T[:, 16:RANKN, :].rearrange("p a b -> p (a b)")
            gt = goutp.tile([P, GGRP, 2, GIDX], i16, name=f"gout_m{m}r{grp}",
                            tag="gout")
            pc = GIDX // 32                              # idx pair-cols per chunk
            for k in range(GGRP):
                g = grp * GGRP + k
                if g < W1CUT:
                    src = T2
                    idxs = idx128[:, g * pc:(g + 1) * pc, :]
                else:
                    src = T2w1
                    idxs = idxw1[:, (g - W1CUT) * pc:(g - W1CUT + 1) * pc, :]
                nc.gpsimd.dma_gather(
                    gt[:, k, :, :], src, idxs.rearrange("p a b -> p (a b)"),
                    num_idxs=GIDX, num_idxs_reg=GIDX, elem_size=256,
                    transpose=True,
                    sbuf_tokens_per_rank=P, sbuf_free_dim_per_rank=512,
                )
            state[m][f"gt{grp}"] = gt

        def emit_outs(m, grp):
            gt = state[m][f"gt{grp}"]
            w = GGRP * GCOLS                             # 1024 fp32 cols per group
            for b in range(NBLK):
                eng = nc.sync if b == 0 else nc.scalar
                eng.dma_start(
                    out_io[m, b * P:(b + 1) * P, grp * w:(grp + 1) * w],
                    gt[:, :, b, :].bitcast(np_f32_dt),
                )

        np_f32_dt = f32

        # software pipeline: mesh 1's load/score overlap mesh 0's select and
        # gather; per-engine emission order is execution order.
        NGRP = GCH // GGRP
        emit_load(0)
        emit_score(0, with_xbar=True)
        emit_select_a(0)
        emit_select_b(0)
        emit_load(1)
        emit_score(1, with_xbar=False)
        emit_select_a(1)
        for grp in range(NGRP):
            emit_gathers(0, grp)
            emit_outs(0, grp)
        nc.scalar.dma_start(nf_io[0:1, :], state[0]["nfs"][:])
        emit_xbar_half(1, 0)
        emit_xbar_half(1, 1)
        emit_select_b(1)
        for grp in range(NGRP):
            emit_gathers(1, grp)
            emit_outs(1, grp)
        nc.scalar.dma_start(nf_io[1:2, :], state[1]["nfs"][:])

    nc.compile()
    return nc


def _host_inputs(x, edges_count):
    x = np.ascontiguousarray(np.asarray(x, dtype=np.float32))
    ec = np.asarray(edges_count).astype(np.int64)
    jj = np.arange(CHUNK)
    iota_g = (np.arange(P) // W0).astype(np.float32).reshape(P, 1)
    grpind = np.zeros((P, 8), np.float32)
    grpind[np.arange(P), np.arange(P) // W0] = 1.0
    t_lev1 = (HIST_LO + iota_g * (HIST_W0 / 8.0)).astype(np.float32)
    f_idx = np.arange(F1)
    iota1w = (f_idx[None, :] * W0 + np.arange(W0)[:, None] + 1).astype(np.float32)
    qq = np.arange(W0)
    perm1 = np.zeros((W0, W0), np.float32)
    perm2 = np.zeros((W0, W0), np.float32)
    for qp in range(W0):
        perm1[qp // 2, qp] = 1.0
        perm2[8 + qp // 2, qp] = 1.0
    rowpar = (qq % 2).astype(np.float32).reshape(W0, 1)
    ones_t = np.ones((P, P), np.float32)

    in_maps = []
    for c in range(NCORES):
        meshes = [c * MPC + m for m in range(MPC)]
        tailm = np.empty((MPC, P, CHUNK), np.float32)
        for m, b in enumerate(meshes):
            tailm[m] = ((TAIL + jj) < ec[b]).astype(np.float32)[None, :]
        in_maps.append({
            "x": x[meshes[0]:meshes[-1] + 1],
            "tailmask": tailm,
            "onesT": ones_t,
            "iota_g": iota_g,
            "grpind": grpind,
            "t_lev1": t_lev1,
            "iota1w": iota1w,
            "perm1": perm1,
            "perm2": perm2,
            "rowpar": rowpar,
        })
    return in_maps


def kernel(x, edges_count, out_channel):
    assert int(out_channel) == K
    if "nc" not in _CACHE:
        _CACHE["nc"] = _build_program()
    nc = _CACHE["nc"]
    in_maps = _host_inputs(x, edges_count)

    from concourse.bass_utils import run_bass_kernel_spmd
    res = run_bass_kernel_spmd(nc, in_maps, list(range(NCORES)))
    _CACHE["last_result"] = res

    out = np.empty((B, C, K), np.float32)
    for c in range(NCORES):
        r = res.results[c]["out"]
        out[c * MPC:(c + 1) * MPC] = np.asarray(r).reshape(MPC, C, K)
        nf = np.asarray(res.results[c]["nf"]).reshape(-1)
        if not (nf == K).all():
            raise RuntimeError(f"core {c}: sparse_gather num_found={nf} != {K}")
    return out


# revision 16
# speedup vs baseline: 1.0399x; 1.0399x over previous
"""MeshPool kernel for Trainium2: per-mesh edge scoring, exact top-K selection,
order-preserving gather.  Data-parallel over B=16 meshes on 8 NeuronCores
(2 meshes per core).

v2 pipeline (replaces the GPSIMD ap_gather backend, ~110us/call, with a
descriptor-DMA gather, ~3us/chunk):

  1. Plain DMA load x -> 2 SBUF tiles [128, 9216] f32 (4 half-tile DMAs).
  2. score[e] = sum_c x[c,e]^2: ACT Square chunks + PE ones-matmul into a
     [1, 512] PSUM strip (fp32 exact); DVE bounces each strip to SBUF and
     Sync DMAs it to a DRAM scratch row score_d[9216].  Tail chunk is
     multiplied by a host 0/1 mask so invalid edges score 0.
  3. Wrapped-16 redistribution: srep[q, f] = score[16f+q]; 16 single-partition
     DMAs read score_d with stride-16; 3 doubling DMAs replicate to [128, 576].
  4. Exact K-th threshold via 7 levels of 8-ary histogram search.  Final bin
     width ~1.5e-5, far under the min adjacent-score gap, so count(>=T) == K.
  5. mask * iota(e+1); GPSIMD sparse_gather compacts to the 4096 kept edges
     (ascending).  The gather needs the DOUBLED interleaved list (2e, 2e+1) in
     wrapped-16 int16 [16, 512]: position 2j+h of the doubled list lives at
     [2*(j%8)+h, 2*(j//16) + (j%16)//8], so rows q'=2u+h pull from sg rows u
     (even cols) and 8+u (odd cols).  Two [16,16] permutation matmuls regroup
     the rows; two DVE adds (+row parity) write even/odd column planes.
  6. XBAR DMA-transpose (u16) of each x block into a token tile T:
     T[p, s*512 + b*256 + 2w] = u16 w of x_b[:, e]-column for 2e+h = 128s+p.
     Token idx = 2e+h sits at partition idx%128, rank idx//128, 512B each.
  7. nc.gpsimd.dma_gather (SBUF-source, transpose): descriptor DMA pulls token
     idx_i and 16-bit-transposes it into dst[:, :, i].  With the interleaved
     (2e, 2e+1) idx list the u16 halves land adjacently, so the dst IS the
     final fp32 [128, 2, cols] output tile: plane 0 = channels 0..127,
     plane 1 = 128..255.  4 chunks of 1024 columns; plain DMAs store each
     chunk straight to DRAM.
"""

import numpy as np

B, C, E, K = 16, 256, 9216, 4096
NCORES = 8
MPC = B // NCORES            # meshes per core
P = 128                      # partitions / channel block
NBLK = C // P                # channel blocks per mesh
CHUNK = 512
NCHUNK = E // CHUNK
TAIL = E - CHUNK             # 8704; all invalid edges live in the last chunk
W0 = 16                      # wrap width (sparse_gather / dma_gather idx ISA)
F1 = E // W0                 # 576 wrapped score columns
SGO = K // W0                # 256 compacted edge columns
SG2 = 2 * K // W0            # 512 doubled idx columns
NIDX = 2 * K                 # 8192 gather indices per mesh
GCH = 16                     # gather chunks per mesh (SWDGE ring caps descs/inst)
GIDX = NIDX // GCH           # 512 indices per gather
GCOLS = K // GCH             # 256 fp32 output columns per gather
GGRP = 2                     # gather chunks per output tile / store group
W1CUT = 12                   # chunks >= W1CUT read the rank-16 window, idx-2048
W1OFF = 2048                 # idx offset of the second window (16 ranks * 128)
RANKN = 2 * E // P           # 144 token ranks (512 B each)
HIST_LO = 240.0              # static threshold bracket; K-th score ~257
HIST_W0 = 32.0               # HIST_HI = 272
NLEV = 6                     # 8-ary levels; final width 32/8^6 ~ 1.2e-4
PIECE = 1536                 # x load piece (3 chunks)

_CACHE = {}


def _build_program():
    import concourse.bacc as bacc
    import concourse.mybir as mybir
    import concourse.tile as tile
    from contextlib import ExitStack

    dt = mybir.dt
    op = mybir.AluOpType
    f32 = dt.float32
    i16 = dt.int16

    nc = bacc.Bacc()

    x_io = nc.dram_tensor("x", [MPC, C, E], f32, kind="ExternalInput")
    tailm_io = nc.dram_tensor("tailmask", [MPC, P, CHUNK], f32, kind="ExternalInput")
    ones_io = nc.dram_tensor("onesT", [P, P], f32, kind="ExternalInput")
    iotag_io = nc.dram_tensor("iota_g", [P, 1], f32, kind="ExternalInput")   # p // 16
    grp_io = nc.dram_tensor("grpind", [P, 8], f32, kind="ExternalInput")     # onehot(p//16)
    t1_io = nc.dram_tensor("t_lev1", [P, 1], f32, kind="ExternalInput")      # lo0+(p//16)*wb0
    iota1_io = nc.dram_tensor("iota1w", [W0, F1], f32, kind="ExternalInput") # 16f+q+1
    perm1_io = nc.dram_tensor("perm1", [W0, W0], f32, kind="ExternalInput")  # p==q'//2
    perm2_io = nc.dram_tensor("perm2", [W0, W0], f32, kind="ExternalInput")  # p==8+q'//2
    rowp_io = nc.dram_tensor("rowpar", [W0, 1], f32, kind="ExternalInput")   # q'%2
    out_io = nc.dram_tensor("out", [MPC, C, K], f32, kind="ExternalOutput")
    nf_io = nc.dram_tensor("nf", [MPC, 1], dt.uint32, kind="ExternalOutput")

    with tile.TileContext(nc) as tc, ExitStack() as ctx:
        constp = ctx.enter_context(tc.tile_pool(name="const", bufs=1))
        xpool = ctx.enter_context(tc.tile_pool(name="xb", bufs=3))
        sqpool = ctx.enter_context(tc.tile_pool(name="sqc", bufs=3))
        psump = ctx.enter_context(tc.tile_pool(name="ps", bufs=6, space="PSUM"))
        psmall = ctx.enter_context(tc.tile_pool(name="psm", bufs=2, space="PSUM"))
        dramp = ctx.enter_context(tc.tile_pool(name="scored", bufs=2, space="DRAM"))
        bouncep = ctx.enter_context(tc.tile_pool(name="bounce", bufs=3))
        tokp = ctx.enter_context(tc.tile_pool(name="tok", bufs=2))
        goutp = ctx.enter_context(tc.tile_pool(name="gout", bufs=2))
        smallp = ctx.enter_context(tc.tile_pool(name="small", bufs=1))

        ones_sb = constp.tile([P, P], f32, name="ones_sb")
        nc.sync.dma_start(ones_sb[:], ones_io[:])
        iotag_sb = constp.tile([P, 1], f32, name="iotag_sb")
        nc.sync.dma_start(iotag_sb[:], iotag_io[:])
        grp_sb = constp.tile([P, 8], f32, name="grp_sb")
        nc.sync.dma_start(grp_sb[:], grp_io[:])
        t1_sb = constp.tile([P, 1], f32, name="t1_sb")
        nc.sync.dma_start(t1_sb[:], t1_io[:])
        iota1_sb = constp.tile([W0, F1], f32, name="iota1_sb")
        nc.sync.dma_start(iota1_sb[:], iota1_io[:])
        perm1_sb = constp.tile([W0, W0], f32, name="perm1_sb")
        nc.sync.dma_start(perm1_sb[:], perm1_io[:])
        perm2_sb = constp.tile([W0, W0], f32, name="perm2_sb")
        nc.sync.dma_start(perm2_sb[:], perm2_io[:])
        rowp_sb = constp.tile([W0, 1], f32, name="rowp_sb")
        nc.sync.dma_start(rowp_sb[:], rowp_io[:])
        tailm_sb = []
        for m in range(MPC):
            tm = constp.tile([P, CHUNK], f32, name=f"tailm_sb{m}")
            nc.sync.dma_start(tm[:], tailm_io[m, :, :])
            tailm_sb.append(tm)

        state = [dict() for _ in range(MPC)]

        NPIECE = E // PIECE          # 6 pieces per block
        CPP = PIECE // CHUNK         # 3 chunks per piece

        def emit_load_piece(m, i):
            # load piece i of both blocks; squares index pieces[blk][i]
            for blk in range(NBLK):
                xt = xpool.tile([P, PIECE], f32, name=f"x_m{m}b{blk}p{i}", tag="xb")
                nc.sync.dma_start(xt[:], x_io[m, blk * P:(blk + 1) * P,
                                               i * PIECE:(i + 1) * PIECE])
                state[m]["pieces"][blk].append(xt)

        def emit_xbar(m):
            # transpose both x blocks straight from DRAM (scalar HWDGE only:
            # sync-issued XBAR corrupts data)
            T = tokp.tile([P, RANKN, 256], i16, name=f"tok_m{m}", tag="tok")
            state[m]["T"] = T
            for blk in range(NBLK):
                nc.scalar.dma_start_transpose(
                    T[:, :, blk * P:(blk + 1) * P],
                    x_io[m, blk * P:(blk + 1) * P, :].bitcast(i16),
                )

        def emit_score(m):
            pieces = state[m]["pieces"]
            score_d = dramp.tile([1, E], f32, name=f"scored_m{m}", tag="sd")
            state[m]["score_d"] = score_d
            LAG = 3

            def drain(ch):
                ps, bnc = state[m]["psq"][ch]
                nc.vector.tensor_copy(bnc[:], ps[:])
                nc.sync.dma_start(score_d[0:1, ch * CHUNK:(ch + 1) * CHUNK], bnc[:])

            state[m]["psq"] = {}
            for ch in range(NCHUNK):
                if ch % CPP == 0:
                    emit_load_piece(m, ch // CPP)
                ps = psump.tile([1, CHUNK], f32, name=f"ps_m{m}c{ch}", tag="ps")
                off = (ch % CPP) * CHUNK
                for blk in range(NBLK):
                    xs = pieces[blk][ch // CPP][:, off:off + CHUNK]
                    sqc = sqpool.tile([P, CHUNK], f32, name=f"sq_m{m}c{ch}b{blk}",
                                      tag="sqc")
                    nc.vector.tensor_tensor(sqc[:], xs, xs, op.mult)
                    if ch == NCHUNK - 1:
                        nc.vector.tensor_tensor(sqc[:], sqc[:], tailm_sb[m][:],
                                                op.mult)
                    nc.tensor.matmul(ps[:], ones_sb[:, 0:1], sqc[:],
                                     start=(blk == 0), stop=(blk == NBLK - 1))
                bnc = bouncep.tile([1, CHUNK], f32, name=f"bnc_m{m}c{ch}", tag="bnc")
                state[m]["psq"][ch] = (ps, bnc)
                if ch >= LAG:
                    drain(ch - LAG)
            for ch in range(NCHUNK - LAG, NCHUNK):
                drain(ch)

        def emit_select_a(m):
            score_d = state[m]["score_d"]
            # wrapped-16 redistribution: srep[q, f] = score[16f+q]
            srep = smallp.tile([P, F1], f32, name=f"srep_m{m}", tag="srep")
            s_w = score_d[0:1, :].rearrange("p (f s) -> p s f", s=W0)  # [1, 16, 576]
            for q in range(W0):
                eng = nc.sync if q % 2 == 0 else nc.scalar
                eng.dma_start(srep[q:q + 1, :], s_w[:, q, :])
            nc.sync.dma_start(srep[W0:2 * W0, :], srep[0:W0, :])
            nc.sync.dma_start(srep[2 * W0:4 * W0, :], srep[0:2 * W0, :])
            nc.sync.dma_start(srep[4 * W0:8 * W0, :], srep[0:4 * W0, :])
            state[m]["srep"] = srep

        def emit_select_b(m):
            srep = state[m]["srep"]
            # 8-ary histogram threshold search; state pair = [lo, wb]
            pair = smallp.tile([1, 2], f32, name=f"pair_m{m}", tag="pair")
            nc.vector.memset(pair[:, 0:1], HIST_LO)
            nc.vector.memset(pair[:, 1:2], HIST_W0 / 8.0)
            ge8 = smallp.tile([P, F1], dt.float8e4, name=f"ge8_m{m}", tag="ge8")
            junk8 = smallp.tile([1, 8], f32, name=f"junk8_m{m}", tag="junk8")
            for lev in range(NLEV):
                if lev == 0:
                    t_ap = t1_sb
                else:
                    tb = psmall.tile([P, 2], f32, name=f"tb_m{m}l{lev}", tag="psm")
                    nc.tensor.matmul(tb[:], ones_sb[0:1, :], pair[:],
                                     start=True, stop=True)
                    t_ap = smallp.tile([P, 1], f32, name=f"tap_m{m}l{lev}", tag="tap")
                    nc.vector.scalar_tensor_tensor(t_ap[:], iotag_sb[:], tb[:, 1:2],
                                                   tb[:, 0:1], op.mult, op.add)
                cnt = smallp.tile([P, 1], f32, name=f"cnt_m{m}l{lev}", tag="cnt")
                nc.vector.tensor_scalar(ge8[:], srep[:], t_ap[:, 0:1], None,
                                        op.is_ge, op1=op.add, accum_out=cnt[:])
                cnt8r = psmall.tile([1, 8], f32, name=f"cnt8_m{m}l{lev}", tag="psm")
                nc.tensor.matmul(cnt8r[:], cnt[:], grp_sb[:], start=True, stop=True)
                # DVE-local tail: s8 = #bins with doubled-count >= 2K (monotone)
                s8 = smallp.tile([1, 1], f32, name=f"s8_m{m}l{lev}", tag="s8")
                nc.vector.tensor_scalar(junk8[:], cnt8r[:], float(K), None,
                                        op.is_ge, op1=op.add, accum_out=s8[:])
                # step = wb*s8 - wb = wb*(s8-1), exact fp32 as in t_g
                step = smallp.tile([1, 1], f32, name=f"step_m{m}l{lev}", tag="step")
                nc.vector.tensor_tensor(step[:], s8[:], pair[:, 1:2], op.mult)
                nc.vector.tensor_tensor(step[:], step[:], pair[:, 1:2], op.subtract)
                nc.vector.tensor_tensor(pair[:, 0:1], pair[:, 0:1], step[:], op.add)
                if lev != NLEV - 1:
                    nc.vector.tensor_scalar(pair[:, 1:2], pair[:, 1:2], 0.125, None,
                                            op.mult)

            # select + compact (doubled sequence)
            t16 = psmall.tile([W0, 1], f32, name=f"t16_m{m}", tag="psm")
            nc.tensor.matmul(t16[:], ones_sb[0:1, 0:W0], pair[:, 0:1],
                             start=True, stop=True)
            m2 = smallp.tile([W0, F1], f32, name=f"m2_m{m}", tag="m2")
            nc.vector.tensor_scalar(m2[:], srep[0:W0, :], t16[:, 0:1], None, op.is_ge)
            nc.vector.tensor_scalar(m2[:], m2[:], 2.0, -1.0, op.mult, op1=op.add)
            sp2 = smallp.tile([W0, F1], f32, name=f"sp2_m{m}", tag="sp2")
            nc.vector.tensor_tensor(sp2[:], m2[:], iota1_sb[:], op.mult)
            sg2 = smallp.tile([W0, SGO], f32, name=f"sg2_m{m}", tag="sg2")
            nfs = smallp.tile([1, 1], dt.uint32, name=f"nfs_m{m}", tag="nfs")
            nc.gpsimd.sparse_gather(sg2[:], sp2[:], num_found=nfs[:])
            # s2 = 2*(e+1) - 2 = 2e; doubled idx list via row-permute matmuls
            s2 = smallp.tile([W0, SGO], f32, name=f"s2_m{m}", tag="s2")
            nc.vector.tensor_scalar(s2[:], sg2[:], 2.0, -2.0, op.mult, op1=op.add)
            pe1 = psmall.tile([W0, SGO], f32, name=f"pe1_m{m}", tag="psm")
            nc.tensor.matmul(pe1[:], perm1_sb[:], s2[:], start=True, stop=True)
            pe2 = psmall.tile([W0, SGO], f32, name=f"pe2_m{m}", tag="psm")
            nc.tensor.matmul(pe2[:], perm2_sb[:], s2[:], start=True, stop=True)
            idx128 = smallp.tile([P, SGO, 2], i16, name=f"idx128_m{m}", tag="idx128")
            nc.vector.tensor_scalar(idx128[0:W0, :, 0:1], pe1[:].rearrange("p (a b) -> p a b", b=1),
                                    rowp_sb[:, 0:1], None, op.add)
            nc.vector.tensor_scalar(idx128[0:W0, :, 1:2], pe2[:].rearrange("p (a b) -> p a b", b=1),
                                    rowp_sb[:, 0:1], None, op.add)
            nc.sync.dma_start(idx128[W0:2 * W0], idx128[0:W0])
            nc.sync.dma_start(idx128[2 * W0:4 * W0], idx128[0:2 * W0])
            nc.sync.dma_start(idx128[4 * W0:8 * W0], idx128[0:4 * W0])
            # second-window idx list (chunks >= W1CUT): value - W1OFF, fp32 path
            ntail = SGO - W1CUT * (GIDX // 32)          # 64 pair-cols
            s2b = smallp.tile([W0, ntail], f32, name=f"s2b_m{m}", tag="s2b")
            nc.vector.tensor_scalar(s2b[:], sg2[:, SGO - ntail:SGO], 2.0,
                                    -2.0 - float(W1OFF), op.mult, op1=op.add)
            pe1b = psmall.tile([W0, ntail], f32, name=f"pe1b_m{m}", tag="psm")
            nc.tensor.matmul(pe1b[:], perm1_sb[:], s2b[:], start=True, stop=True)
            pe2b = psmall.tile([W0, ntail], f32, name=f"pe2b_m{m}", tag="psm")
            nc.tensor.matmul(pe2b[:], perm2_sb[:], s2b[:], start=True, stop=True)
            idxw1 = smallp.tile([P, ntail, 2], i16, name=f"idxw1_m{m}", tag="idxw1")
            nc.vector.tensor_scalar(idxw1[0:W0, :, 0:1],
                                    pe1b[:].rearrange("p (a b) -> p a b", b=1),
                                    rowp_sb[:, 0:1], None, op.add)
            nc.vector.tensor_scalar(idxw1[0:W0, :, 1:2],
                                    pe2b[:].rearrange("p (a b) -> p a b", b=1),
                                    rowp_sb[:, 0:1], None, op.add)
            nc.sync.dma_start(idxw1[W0:2 * W0], idxw1[0:W0])
            nc.sync.dma_start(idxw1[2 * W0:4 * W0], idxw1[0:2 * W0])
            nc.sync.dma_start(idxw1[4 * W0:8 * W0], idxw1[0:4 * W0])
            state[m]["idx128"] = idx128
            state[m]["idxw1"] = idxw1
            state[m]["nfs"] = nfs

        def emit_gathers(m, grp):
            # chunks [grp*GGRP, (grp+1)*GGRP) into one chunk-major tile
            T = state[m]["T"]
            idx128 = state[m]["idx128"]
            idxw1 = state[m]["idxw1"]
            T2 = T[:].rearrange("p a b -> p (a b)")
            T2w1 = T[:, 16:RANKN, :].rearrange("p a b -> p (a b)")
            gt = goutp.tile([P, GGRP, 2, GIDX], i16, name=f"gout_m{m}r{grp}",
                            tag="gout")
            pc = GIDX // 32                              # idx pair-cols per chunk
            for k in range(GGRP):
                g = grp * GGRP + k
                if g < W1CUT:
                    src = T2
                    idxs = idx128[:, g * pc:(g + 1) * pc, :]
                else:
                    src = T2w1
                    idxs = idxw1[:, (g - W1CUT) * pc:(g - W1CUT + 1) * pc, :]
                nc.gpsimd.dma_gather(
                    gt[:, k, :, :], src, idxs.rearrange("p a b -> p (a b)"),
                    num_idxs=GIDX, num_idxs_reg=GIDX, elem_size=256,
                    transpose=True,
                    sbuf_tokens_per_rank=P, sbuf_free_dim_per_rank=512,
                )
            state[m][f"gt{grp}"] = gt

        def emit_outs(m, grp):
            gt = state[m][f"gt{grp}"]
            w = GGRP * GCOLS                             # 1024 fp32 cols per group
            for b in range(NBLK):
                eng = nc.sync if b == 0 else nc.scalar
                eng.dma_start(
                    out_io[m, b * P:(b + 1) * P, grp * w:(grp + 1) * w],
                    gt[:, :, b, :].bitcast(np_f32_dt),
                )

        np_f32_dt = f32

        # software pipeline: mesh 1's load/score overlap mesh 0's select and
        # gather; per-engine emission order is execution order.
        NGRP = GCH // GGRP
        state[0]["pieces"] = [[], []]
        state[1]["pieces"] = [[], []]
        emit_xbar(0)
        emit_xbar(1)
        emit_score(0)
        emit_select_a(0)
        emit_select_b(0)
        emit_score(1)
        emit_select_a(1)
        for grp in range(NGRP):
            emit_gathers(0, grp)
            emit_outs(0, grp)
        nc.scalar.dma_start(nf_io[0:1, :], state[0]["nfs"][:])
        emit_select_b(1)
        for grp in range(NGRP):
            emit_gathers(1, grp)
            emit_outs(1, grp)
        nc.scalar.dma_start(nf_io[1:2, :], state[1]["nfs"][:])

    nc.compile()
    return nc


def _host_inputs(x, edges_count):
    x = np.ascontiguousarray(np.asarray(x, dtype=np.float32))
    ec = np.asarray(edges_count).astype(np.int64)
    jj = np.arange(CHUNK)
    iota_g = (np.arange(P) // W0).astype(np.float32).reshape(P, 1)
    grpind = np.zeros((P, 8), np.float32)
    grpind[np.arange(P), np.arange(P) // W0] = 1.0
    t_lev1 = (HIST_LO + iota_g * (HIST_W0 / 8.0)).astype(np.float32)
    f_idx = np.arange(F1)
    iota1w = (f_idx[None, :] * W0 + np.arange(W0)[:, None] + 1).astype(np.float32)
    qq = np.arange(W0)
    perm1 = np.zeros((W0, W0), np.float32)
    perm2 = np.zeros((W0, W0), np.float32)
    for qp in range(W0):
        perm1[qp // 2, qp] = 1.0
        perm2[8 + qp // 2, qp] = 1.0
    rowpar = (qq % 2).astype(np.float32).reshape(W0, 1)
    ones_t = np.ones((P, P), np.float32)

    in_maps = []
    for c in range(NCORES):
        meshes = [c * MPC + m for m in range(MPC)]
        tailm = np.empty((MPC, P, CHUNK), np.float32)
        for m, b in enumerate(meshes):
            tailm[m] = ((TAIL + jj) < ec[b]).astype(np.float32)[None, :]
        in_maps.append({
            "x": x[meshes[0]:meshes[-1] + 1],
            "tailmask": tailm,
            "onesT": ones_t,
            "iota_g": iota_g,
            "grpind": grpind,
            "t_lev1": t_lev1,
            "iota1w": iota1w,
            "perm1": perm1,
            "perm2": perm2,
            "rowpar": rowpar,
        })
    return in_maps


def kernel(x, edges_count, out_channel):
    assert int(out_channel) == K
    if "nc" not in _CACHE:
        _CACHE["nc"] = _build_program()
    nc = _CACHE["nc"]
    in_maps = _host_inputs(x, edges_count)

    from concourse.bass_utils import run_bass_kernel_spmd
    res = run_bass_kernel_spmd(nc, in_maps, list(range(NCORES)))
    _CACHE["last_result"] = res

    out = np.empty((B, C, K), np.float32)
    for c in range(NCORES):
        r = res.results[c]["out"]
        out[c * MPC:(c + 1) * MPC] = np.asarray(r).reshape(MPC, C, K)
        nf = np.asarray(res.results[c]["nf"]).reshape(-1)
        if not (nf == K).all():
            raise RuntimeError(f"core {c}: sparse_gather num_found={nf} != {K}")
    return out
